# revision 1
# baseline (speedup 1.0000x reference)
"""Multi-head attention (B=2, T=2048, D=2048, H=16, HD=128) on 8 Trainium2
NeuronCores.

Sharding: core c in 0..7 handles batch b = c // 4 and head group g = c % 4
(4 heads per core, tensor-parallel over heads within each batch element).
wq/wk/wv are column-sharded, wo row-sharded; the per-head-group partial
outputs are summed on the host, then the two batch elements are stacked.

All matmul operands are fp16: 1 cycle/row on the PE (f32r moving
operands measured ~2 cycles/row on this hardware, so tf32 loses despite
its self-loading stationaries), half the DMA/SBUF of f32, and 10 mantissa
bits keep the end-to-end rel err at ~7e-4 (gate 2e-2). PSUM accumulation
stays fp32.

Device kernel (per core, SPMD):
  phase A: single streaming pass over x in t-halves; per half the
           roped K and Q head-tiles (hd on partitions, [128,1024] 2-bank
           PSUM accumulators) and the V tiles (t on partitions) are
           produced with wk/wq/wv resident. RoPE
           pairs are made partition-contiguous by permuting the wq/wk
           output rows per head on the host (even hd components in
           partitions 0..63, odd in 64..127), which turns the rotation
           into an ACT stage copy + two Pool half-mults (host-swapped
           [sin;-sin] table keeps SBUF input base partitions equal, which
           walrus requires) + two DVE 2x-mode ops.
  phase B: per (q-chunk, head): scoresT = KT_k.T @ QT (keys on partitions,
           q free), exp on ACT over k-tile PAIRS ([128,1024] activations,
           softmax scale folded into the activation; scores are O(5) so no
           max subtraction), softmax denominators as a 4-op DVE fp16
           binary tree over the exp row (HW DVE op latency ~1.2us makes
           15-deep serial chains expensive) + a gpsimd
           partition_all_reduce + DVE reciprocal, unnormalized
           out = V.T-matmuls with exp tiles moving, normalization via
           DVE multiply.
  phase C: partial_y = aoT.T @ woT accumulated over the 4 head k-steps.
           C chains of q-chunk qc-1 are interleaved between phase-B heads
           of chunk qc: phase B alone is ACT(exp)-paced, so the PE bubbles
           are filled with C matmuls.
"""
from contextlib import ExitStack

import numpy as np

B, T, D, H = 2, 2048, 2048, 16
HD = D // H            # 128
N_CORES = 8
HPC = H // 4           # 4 heads per core
JC = HPC * HD          # 512 per-core projection width
KD = D // 128          # 16 contraction tiles for the projections
TQ = 512               # matmul moving-operand width cap
TH = 1024              # phase-A t-half-of-half width (2-bank PSUM tiles)
N_TH = T // TH         # 2
QC = 512               # q-chunk width in phase B
N_QC = T // QC         # 4
KT_TILES = T // 128    # 16 key tiles

import os as _os

# analysis aid: truncate the program after phase a / b (default: full)
PHASES = _os.environ.get("K_PHASES", "full")
PSA_BUFS = int(_os.environ.get("K_PSA_BUFS", "4"))
RT_BUFS = int(_os.environ.get("K_RT_BUFS", "6"))
PT_BUFS = int(_os.environ.get("K_PT_BUFS", "2"))
SC_BUFS = int(_os.environ.get("K_SC_BUFS", "2"))
PC_BUFS = int(_os.environ.get("K_PC_BUFS", "2"))

_cache = {}


def _build_program():
    import concourse.bacc as bacc
    import concourse.tile as tile
    from concourse import mybir

    F16 = mybir.dt.float16
    F32 = mybir.dt.float32
    F32R = mybir.dt.float32r
    AF = mybir.ActivationFunctionType
    ALU = mybir.AluOpType

    nc = bacc.Bacc("TRN2", target_bir_lowering=False, debug=False,
                   num_devices=N_CORES)

    xT = nc.dram_tensor("xT", [D, T], F16, kind="ExternalInput").ap()
    wqT = nc.dram_tensor("wqT", [D, JC], F16, kind="ExternalInput").ap()
    wkT = nc.dram_tensor("wkT", [D, JC], F16, kind="ExternalInput").ap()
    wvT = nc.dram_tensor("wvT", [D, JC], F16, kind="ExternalInput").ap()
    woT = nc.dram_tensor("woT", [JC, D], F16, kind="ExternalInput").ap()
    csA = nc.dram_tensor("csA", [128, T], F16, kind="ExternalInput").ap()
    csB = nc.dram_tensor("csB", [128, T], F16, kind="ExternalInput").ap()
    ones1 = nc.dram_tensor("ones1", [128, 1], F16, kind="ExternalInput").ap()
    ones2 = nc.dram_tensor("ones2", [1, 128], F32R, kind="ExternalInput").ap()
    py = nc.dram_tensor("py", [T, D], F16, kind="ExternalOutput").ap()

    sc_scale = float(HD) ** -0.5

    with tile.TileContext(nc) as tc, ExitStack() as ctx:
        p_big = ctx.enter_context(tc.tile_pool(name="big", bufs=1,
                                               side="right"))
        KT = [p_big.tile([128, T], F16, tag=f"KT{h}", name=f"KT{h}")
              for h in range(HPC)]
        QT = [p_big.tile([128, T], F16, tag=f"QT{h}", name=f"QT{h}")
              for h in range(HPC)]
        V = p_big.tile([128, KT_TILES * JC], F16, tag="V", name="V")

        # ---- phase A: projections, single pass over x ----
        with tc.tile_pool(name="cs", bufs=1) as p_cs, \
             tc.tile_pool(name="w3", bufs=1) as p_w, \
             tc.tile_pool(name="xa", bufs=2) as p_x, \
             tc.tile_pool(name="ropetmp", bufs=RT_BUFS) as p_rt, \
             tc.tile_pool(name="psA", bufs=3, space="PSUM") as psA, \
             tc.tile_pool(name="psAV", bufs=2, space="PSUM") as psAV:

            csa_t = p_cs.tile([128, T], F16, tag="csa")
            csb_t = p_cs.tile([128, T], F16, tag="csb")
            wk_t = p_w.tile([128, KD * JC], F16, tag="wk", name="wk")
            wq_t = p_w.tile([128, KD * JC], F16, tag="wq", name="wq")
            wv_t = p_w.tile([128, KD * JC], F16, tag="wv", name="wv")

            KCH = int(_os.environ.get("K_DMA_KC", "1"))  # k-tiles per DMA

            def load_xq(e, xte=None):
                if xte is None:
                    xte = p_x.tile([128, KD * TH], F16, tag="xte")
                for k0 in range(0, KD, KCH):
                    nc.sync.dma_start(
                        xte[:, k0 * TH:(k0 + KCH) * TH].rearrange(
                            "p (k t) -> p k t", k=KCH),
                        xT[k0 * 128:(k0 + KCH) * 128,
                           e * TH:(e + 1) * TH].rearrange(
                            "(k p) t -> p k t", p=128))
                return xte

            def load_w(wt, dram, k0, kc):
                nc.sync.dma_start(
                    wt[:, k0 * JC:(k0 + kc) * JC].rearrange(
                        "p (k j) -> p k j", k=kc),
                    dram[k0 * 128:(k0 + kc) * 128, :].rearrange(
                        "(k p) j -> p k j", p=128))

            # emission order = DMA drain order: interleave the first x
            # quarter with wk (first compute), then cs (first rope), wq, wv
            xte0 = p_x.tile([128, KD * TH], F16, tag="xte", name="xte0")
            for k0 in range(0, KD, KCH):
                nc.sync.dma_start(
                    xte0[:, k0 * TH:(k0 + KCH) * TH].rearrange(
                        "p (k t) -> p k t", k=KCH),
                    xT[k0 * 128:(k0 + KCH) * 128, 0:TH].rearrange(
                        "(k p) t -> p k t", p=128))
                load_w(wk_t, wkT, k0, KCH)
            nc.sync.dma_start(csa_t[:], csA[:])
            nc.sync.dma_start(csb_t[:], csB[:])
            for k0 in range(0, KD, KCH):
                load_w(wq_t, wqT, k0, KCH)
            for k0 in range(0, KD, KCH):
                load_w(wv_t, wvT, k0, KCH)

            def rope(ps_tile, dst, t0):
                # stage PSUM -> SBUF fp16 on ACT (idle in phase A); with the
                # host-swapped sin table ([sin;-sin]) both SBUF inputs of
                # each half-mult share a base partition (walrus NCC_IBIR297),
                # so the halves can run on Pool and u/add on DVE in 2x mode
                st = p_rt.tile([128, TH], F16, tag="ropest")
                nc.scalar.copy(st[:], ps_tile[:])
                u = p_rt.tile([128, TH], F16, tag="ropeu")
                v = p_rt.tile([128, TH], F16, tag="ropev")
                nc.vector.tensor_tensor(u[:], st[:],
                                        csa_t[:, t0:t0 + TH], ALU.mult)
                nc.gpsimd.tensor_tensor(v[0:64, :], st[64:128, :],
                                        csb_t[64:128, t0:t0 + TH], ALU.mult)
                nc.gpsimd.tensor_tensor(v[64:128, :], st[0:64, :],
                                        csb_t[0:64, t0:t0 + TH], ALU.mult)
                nc.vector.tensor_tensor(dst[:, t0:t0 + TH], u[:], v[:],
                                        ALU.add)

            for e in range(N_TH):
                xte = xte0 if e == 0 else load_xq(e)
                # all K chains first: wq/wv may still be in flight on the
                # first half. qk tiles are [128,1024] spanning 2 PSUM banks
                # (each sub-chain writes one bank) so one rope application
                # covers 1024 columns -> half the per-op overhead.
                for wt, dst in ((wk_t, KT), (wq_t, QT)):
                    for j in range(HPC):
                        acc = psA.tile([128, TH], F32, tag="qk")
                        for q2 in range(TH // TQ):
                            for k in range(KD):
                                nc.tensor.matmul(
                                    acc[:, q2 * TQ:(q2 + 1) * TQ],
                                    wt[:, k * JC + j * 128:
                                       k * JC + (j + 1) * 128],
                                    xte[:, k * TH + q2 * TQ:
                                        k * TH + (q2 + 1) * TQ],
                                    start=(k == 0), stop=(k == KD - 1))
                        rope(acc, dst[j], e * TH)
                for tl in range(TH // 128):
                    acc = psAV.tile([128, JC], F32, tag="v")
                    for k in range(KD):
                        nc.tensor.matmul(
                            acc[:],
                            xte[:, k * TH + tl * 128:k * TH + (tl + 1) * 128],
                            wv_t[:, k * JC:(k + 1) * JC],
                            start=(k == 0), stop=(k == KD - 1))
                    tt = e * (TH // 128) + tl
                    nc.scalar.copy(V[:, tt * JC:(tt + 1) * JC], acc[:])

        # ---- phases B + C (interleaved) ----
        if PHASES != "a":
            with tc.tile_pool(name="wo", bufs=1) as p_wo, \
                 tc.tile_pool(name="pt", bufs=PT_BUFS) as p_pt, \
                 tc.tile_pool(name="ds", bufs=2) as p_ds, \
                 tc.tile_pool(name="ao", bufs=8) as p_ao, \
                 tc.tile_pool(name="bmisc", bufs=4) as p_bm, \
                 tc.tile_pool(name="pyout", bufs=6) as p_po, \
                 tc.tile_pool(name="psSC", bufs=SC_BUFS, space="PSUM") as psSC, \
                 tc.tile_pool(name="psOU", bufs=int(_os.environ.get("K_OU_BUFS", "1")), space="PSUM") as psOU, \
                 tc.tile_pool(name="psC", bufs=PC_BUFS, space="PSUM") as psC:

                from concourse.bass_isa import ReduceOp

                wo_t = p_wo.tile([128, HPC * D], F16, tag="wo")
                nc.sync.dma_start(
                    wo_t[:].rearrange("p (j e) -> p j e", j=HPC),
                    woT[:].rearrange("(j p) e -> p j e", p=128))

                def phase_c_chains(qc, ao, tl):
                    """Emit phase-C chains of q-chunk qc for t-block tl."""
                    ts_ = qc * QC + tl * 128
                    # ec pairs share the ao[j] stationary loads
                    for e0 in range(0, D // 512, PC_BUFS):
                        accs = [psC.tile([128, 512], F32, tag="py",
                                         name=f"pyacc{i}")
                                for i in range(PC_BUFS)]
                        for j in range(HPC):
                            for i, acc in enumerate(accs):
                                ec = e0 + i
                                nc.tensor.matmul(
                                    acc[:],
                                    ao[j][:, tl * 128:(tl + 1) * 128],
                                    wo_t[:, j * D + ec * 512:
                                         j * D + (ec + 1) * 512],
                                    start=(j == 0), stop=(j == HPC - 1))
                        # both ec drains share one staging tile and one
                        # output DMA (halves the serial HWDGE issue count)
                        out_sb = p_po.tile([128, 1024], F16, tag="pyo")
                        for i, acc in enumerate(accs):
                            # spread the PSUM->SBUF drains: 1 in 4 on ACT
                            if e0 + i == 0:
                                nc.scalar.copy(out_sb[:, i * 512:
                                                      (i + 1) * 512], acc[:])
                            else:
                                nc.vector.tensor_copy(
                                    out_sb[:, i * 512:(i + 1) * 512], acc[:])
                        nc.sync.dma_start(
                            py[ts_:ts_ + 128, e0 * 512:(e0 + 2) * 512],
                            out_sb[:])

                prev = None  # (qc, ao list) whose phase C is pending
                for qc in range(N_QC):
                    qs = qc * QC
                    ao = []
                    for h in range(HPC):
                        pt = p_pt.tile([128, KT_TILES * QC], F16, tag="pt")
                        dsum = p_ds.tile([128, QC], F16, tag="ds")
                        # exp over k-tile pairs: [128,1024] activations
                        # halve the per-instruction ACT overhead
                        for k2 in range(KT_TILES // 2):
                            sc = psSC.tile([128, 2 * QC], F32, tag="sc")
                            for i in range(2):
                                k = 2 * k2 + i
                                nc.tensor.matmul(
                                    sc[:, i * QC:(i + 1) * QC],
                                    KT[h][:, k * 128:(k + 1) * 128],
                                    QT[h][:, qs:qs + QC],
                                    start=True, stop=True)
                            nc.scalar.activation(
                                pt[:, 2 * k2 * QC:(2 * k2 + 2) * QC], sc[:],
                                AF.Exp, scale=sc_scale)

                        # softmax denominator: 4-op binary tree over the
                        # full pt row (breaks the serial add chain; DVE op
                        # latency on HW is ~1.2us, so depth matters)
                        dt1 = p_ds.tile([128, 8 * QC], F16, tag="dt1")
                        nc.vector.tensor_tensor(
                            dt1[:], pt[:, 0:8 * QC], pt[:, 8 * QC:16 * QC],
                            ALU.add)
                        nc.vector.tensor_tensor(
                            dt1[:, 0:4 * QC], dt1[:, 0:4 * QC],
                            dt1[:, 4 * QC:8 * QC], ALU.add)
                        nc.vector.tensor_tensor(
                            dt1[:, 0:2 * QC], dt1[:, 0:2 * QC],
                            dt1[:, 2 * QC:4 * QC], ALU.add)
                        nc.vector.tensor_tensor(
                            dsum[:], dt1[:, 0:QC], dt1[:, QC:2 * QC],
                            ALU.add)
                        ou = psOU.tile([128, QC], F32, tag="ou")
                        for k in range(KT_TILES):
                            nc.tensor.matmul(
                                ou[:],
                                V[:, k * JC + h * 128:k * JC + (h + 1) * 128],
                                pt[:, k * QC:(k + 1) * QC],
                                start=(k == 0), stop=(k == KT_TILES - 1))
                        # softmax denominator: all-reduce over partitions on
                        # the (otherwise idle) gpsimd engine, then reciprocal
                        db = p_bm.tile([128, QC], F32R, tag="db")
                        nc.gpsimd.partition_all_reduce(db[:], dsum[:], 128,
                                                       ReduceOp.add)
                        rcb = p_bm.tile([128, QC], F32R, tag="rcb")
                        with nc.allow_low_precision(reason="softmax denom tf32"):
                            nc.vector.reciprocal(rcb[:], db[:])
                        ao_h = p_ao.tile([128, QC], F16, tag="ao")
                        nc.vector.tensor_tensor(ao_h[:], ou[:], rcb[:],
                                                ALU.mult)
                        ao.append(ao_h)
                        # fill PE bubbles (phase B is ACT-paced) with one
                        # t-block of the previous chunk's phase C
                        if prev is not None and PHASES != "b":
                            phase_c_chains(prev[0], prev[1], h)
                    prev = (qc, ao)
                if PHASES != "b":
                    for tl in range(QC // 128):
                        phase_c_chains(prev[0], prev[1], tl)

    nc.compile()
    return nc


def _prep_inputs(x, freqs_cis, wq, wk, wv, wo):
    """Host-side shard + layout prep. Returns in_maps for the 8 cores."""
    # even/odd permutation within each head's 128 rows (rope pairs ->
    # partition halves)
    perm = np.concatenate([np.arange(0, HD, 2), np.arange(1, HD, 2)])

    cos = np.ascontiguousarray(freqs_cis[:, :, 0].T, dtype=np.float32)  # (64,T)
    sin = np.ascontiguousarray(freqs_cis[:, :, 1].T, dtype=np.float32)
    csA = np.concatenate([cos, cos], axis=0).astype(np.float16)   # (128, T)
    csB = np.concatenate([sin, -sin], axis=0).astype(np.float16)  # (128, T), pre-swapped
    ones1 = np.ones((128, 1), np.float16)
    ones2 = np.ones((1, 128), np.float32)

    in_maps = []
    for c in range(N_CORES):
        b, g = divmod(c, 4)
        rows = slice(g * JC, (g + 1) * JC)
        wq_g = wq[rows].reshape(HPC, HD, D)[:, perm].reshape(JC, D)
        wk_g = wk[rows].reshape(HPC, HD, D)[:, perm].reshape(JC, D)
        wv_g = wv[rows]
        wo_g = wo[:, rows]
        in_maps.append({
            "xT": np.ascontiguousarray(x[b].T).astype(np.float16),
            "wqT": np.ascontiguousarray(wq_g.T).astype(np.float16),
            "wkT": np.ascontiguousarray(wk_g.T).astype(np.float16),
            "wvT": np.ascontiguousarray(wv_g.T).astype(np.float16),
            "woT": np.ascontiguousarray(wo_g.T).astype(np.float16),
            "csA": csA,
            "csB": csB,
            "ones1": ones1,
            "ones2": ones2,
        })
    return in_maps


def _make_runner(nc):
    """Cacheable jitted SPMD runner (mirrors bass2jax.run_bass_via_pjrt's
    multi-core path, minus donation, so one jit serves repeated calls)."""
    import jax
    from concourse import mybir
    from concourse.bass2jax import (
        _bass_exec_p, install_neuronx_cc_hook, partition_id_tensor)
    from jax.experimental.shard_map import shard_map
    from jax.sharding import Mesh, NamedSharding, PartitionSpec

    install_neuronx_cc_hook()
    partition_name = (
        nc.partition_id_tensor.name if nc.partition_id_tensor else None)
    in_names, out_names, out_avals, zero_outs = [], [], [], []
    for alloc in nc.m.functions[0].allocations:
        if not isinstance(alloc, mybir.MemoryLocationSet):
            continue
        name = alloc.memorylocations[0].name
        if alloc.kind == "ExternalInput":
            if name != partition_name:
                in_names.append(name)
        elif alloc.kind == "ExternalOutput":
            out_names.append(name)
            shape = tuple(alloc.tensor_shape)
            dtype = mybir.dt.np(alloc.dtype)
            out_avals.append(jax.core.ShapedArray(shape, dtype))
            zero_outs.append(np.zeros(shape, dtype))
    all_in_names = list(in_names) + out_names
    if partition_name is not None:
        all_in_names.append(partition_name)

    def _body(*args):
        operands = list(args)
        if partition_name is not None:
            operands.append(partition_id_tensor())
        outs = _bass_exec_p.bind(
            *operands,
            out_avals=tuple(out_avals),
            in_names=tuple(all_in_names),
            out_names=tuple(out_names),
            lowering_input_output_aliases=(),
            sim_require_finite=True,
            sim_require_nnan=True,
            nc=nc,
        )
        return tuple(outs)

    devices = jax.devices()[:N_CORES]
    assert len(devices) == N_CORES, f"need {N_CORES} devices, got {devices}"
    mesh = Mesh(np.asarray(devices), ("core",))
    nshard = NamedSharding(mesh, PartitionSpec("core"))
    n_in = len(in_names) + len(out_names)
    jf = jax.jit(
        shard_map(_body, mesh=mesh,
                  in_specs=(PartitionSpec("core"),) * n_in,
                  out_specs=(PartitionSpec("core"),) * len(out_names),
                  check_rep=False),
        keep_unused=True,
    )
    dev_zero = [
        jax.device_put(
            np.zeros((N_CORES * z.shape[0], *z.shape[1:]), z.dtype), nshard)
        for z in zero_outs
    ]

    def run(in_maps):
        concat_in = [
            np.concatenate([np.asarray(in_maps[c][nm])
                            for c in range(N_CORES)], axis=0)
            for nm in in_names
        ]
        dev_in = [jax.device_put(a, nshard) for a in concat_in]
        outs = jf(*dev_in, *dev_zero)
        return {
            name: np.asarray(outs[i]) for i, name in enumerate(out_names)
        }

    return run


def kernel(x, freqs_cis, wq, wk, wv, wo):
    if "nc" not in _cache:
        _cache["nc"] = _build_program()
    if "run" not in _cache:
        _cache["run"] = _make_runner(_cache["nc"])

    in_maps = _prep_inputs(
        np.asarray(x), np.asarray(freqs_cis), np.asarray(wq),
        np.asarray(wk), np.asarray(wv), np.asarray(wo))
    outs = _cache["run"](in_maps)
    pys = outs["py"].reshape(N_CORES, T, D).astype(np.float64)

    out = np.empty((B, T, D), dtype=np.float32)
    for b in range(B):
        acc = pys[b * 4]
        for g in range(1, 4):
            acc = acc + pys[b * 4 + g]
        out[b] = acc.astype(np.float32)
    return out

